# revision 25
# baseline (speedup 1.0000x reference)
"""Causal self-attention (GPT-style, 12 heads, C=768) on 8 TRN2 NeuronCores.

Sharding: core c -> (batch b = c//2, head-group g = c%2 of 6 heads).
Each core computes qkv projection for its 6 heads, causal attention, and a
partial output projection (its 384 rows of w_proj). Host sums the two
partial projections per batch (row-parallel tensor parallelism) and adds
nothing else (b_proj is folded into the g=0 core's partial).

All matmuls run as float32r (full PE rate at N>=256, ~1e-4 precision).
Layouts chosen so no on-device transposes are needed:
  - x is transposed on host -> xT [C, T]
  - qkv matmul produces qT/kT directly ([head-pair d, T]); V in natural [T, d]
  - scores computed transposed: sT[j, i] = K Q^T via lhsT=kT, rhs=qT
  - softmax denominator via ones-vector matmul (S = sum_j exp)
  - out^T [d, i] = V^T exp accumulated in PSUM, normalized by 1/S broadcast
    (broadcast via K=1 matmul), written as outT [384, T] = proj lhsT.
Head-pairs are packed 2-per-128-partitions: QK uses row-tiled concurrent
matmuls (K=64 at base partition 0/64), AV/S use col-tiled concurrent
matmuls (output at partition base 0/64 resp. 0/32).
"""

import numpy as np

import concourse.bass as bass
import concourse.mybir as mybir
import concourse.tile as tile
from concourse import bacc
from concourse import bass_utils

f32 = mybir.dt.float32
f32r = mybir.dt.float32r
AF = mybir.ActivationFunctionType
ALU = mybir.AluOpType

N_HEAD = 12
N_EMBD = 768
B_FULL = 4
T_FULL = 2048
N_CORES = 8
SCALE = float(N_EMBD) ** -0.5

TRACE = False
LAST_RESULT = None
_NC_CACHE = {}


def build_nc(T=T_FULL, dbg=False):
    """Build the per-core Bass program. All 8 cores run this same program
    on different input data."""
    C = N_EMBD            # 768
    HC = 6                # local heads per core
    NP = 3                # head pairs
    D = 64                # head dim
    KT = C // 128         # 6 k-tiles for the projections
    NIC = T // 512        # i-chunks (512 queries each)
    NJT = T // 128        # j-tiles (128 keys each)

    nc = bacc.Bacc("TRN2", target_bir_lowering=False, debug=False)

    xT_d = nc.dram_tensor("xT", [C, T], f32r, kind="ExternalInput")
    wqk_d = nc.dram_tensor("wqk", [C, 768], f32r, kind="ExternalInput")
    wv_d = nc.dram_tensor("wv", [C, 384], f32r, kind="ExternalInput")
    wp_d = nc.dram_tensor("wp", [384, C], f32r, kind="ExternalInput")
    bqk_d = nc.dram_tensor("bqk", [6, 128, 1], f32, kind="ExternalInput")
    bv_d = nc.dram_tensor("bv", [1, 384], f32r, kind="ExternalInput")
    bp_d = nc.dram_tensor("bp", [1, C], f32r, kind="ExternalInput")
    ones_d = nc.dram_tensor("ones", [128, 128], f32r, kind="ExternalInput")
    mask_d = nc.dram_tensor("mask", [128, 4, 512], f32r, kind="ExternalInput")
    y_d = nc.dram_tensor("y", [T, C], f32, kind="ExternalOutput")
    if dbg:
        dbg_qT = nc.dram_tensor("dbg_qT", [128, T], f32r, kind="ExternalOutput")
        dbg_kT = nc.dram_tensor("dbg_kT", [128, T], f32r, kind="ExternalOutput")
        dbg_v = nc.dram_tensor("dbg_v", [128, 390], f32r, kind="ExternalOutput")
        dbg_oT = nc.dram_tensor("dbg_oT", [128, T], f32r, kind="ExternalOutput")
        dbg_ef = nc.dram_tensor("dbg_ef", [128, 1024], f32r, kind="ExternalOutput")
        dbg_ed = nc.dram_tensor("dbg_ed", [128, 512], f32r, kind="ExternalOutput")
        dbg_av = nc.dram_tensor("dbg_av", [128, 512], f32, kind="ExternalOutput")
        dbg_s = nc.dram_tensor("dbg_s", [2, 512], f32, kind="ExternalOutput")
        dbg_rb = nc.dram_tensor("dbg_rb", [128, 512], f32, kind="ExternalOutput")

    with tile.TileContext(nc) as tc:
        with (
            tc.tile_pool(name="const", bufs=1) as constp,
            tc.tile_pool(name="xt", bufs=8) as xtp,
            tc.tile_pool(name="qk", bufs=1) as qkp,
            tc.tile_pool(name="vs", bufs=16) as vsp,
            tc.tile_pool(name="es", bufs=3) as esp,
            tc.tile_pool(name="ot", bufs=1) as otp,
            tc.tile_pool(name="ys", bufs=2) as ysp,
            tc.tile_pool(name="rs", bufs=1) as rsp,
            tc.tile_pool(name="psg", bufs=2, space="PSUM") as psgp,
            tc.tile_pool(name="pav", bufs=2, space="PSUM") as pavp,
        ):
            # ---------------- setup: only what the qkv phase needs -------
            ones = constp.tile([128, 128], f32r, tag="ones")
            nc.sync.dma_start(ones[:], ones_d.ap()[:])
            wqk = []
            wv = []
            for k in range(KT):
                t = constp.tile([128, 768], f32r, tag=f"wqk{k}")
                nc.sync.dma_start(t[:], wqk_d.ap()[128 * k:128 * (k + 1), :])
                wqk.append(t)
                t = constp.tile([128, 384], f32r, tag=f"wv{k}")
                nc.sync.dma_start(t[:], wv_d.ap()[128 * k:128 * (k + 1), :])
                wv.append(t)
            bqk = []
            for m in range(6):
                t = constp.tile([128, 1], f32, tag=f"bqk{m}")
                nc.sync.dma_start(t[:], bqk_d.ap()[m, :, :])
                bqk.append(t)
            bv_row = constp.tile([1, 384], f32r, tag="bvr")
            nc.sync.dma_start(bv_row[:], bv_d.ap()[:])
            bvb = constp.tile([128, 384], f32, tag="bvb")
            ps = psgp.tile([128, 384], f32, tag="sg")
            nc.tensor.matmul(ps[:], ones[0:1, :], bv_row[:], start=True, stop=True)
            nc.vector.tensor_copy(bvb[:], ps[:])

            # ---------------- qkv projection ------------------------------
            # qT/kT: [128 (pair dims), T]; v: per t-tile [128 (t), 384]
            qT = [qkp.tile([128, T], f32r, tag=f"qT{p}", name=f"qT{p}") for p in range(NP)]
            kT = [qkp.tile([128, T], f32r, tag=f"kT{p}", name=f"kT{p}") for p in range(NP)]
            v = [vsp.tile([128, 6, 65], f32r, tag="v", name=f"v{j}") for j in range(NJT)]
            for j in range(NJT):
                nc.sync.dma_start(v[j][:, :, 64:65], ones[:, 0:6])

            for tci in range(NIC):
                ts512 = slice(512 * tci, 512 * (tci + 1))
                xts = []
                for k in range(KT):
                    xt = xtp.tile([128, 512], f32r, tag="xt")
                    nc.sync.dma_start(xt[:], xT_d.ap()[128 * k:128 * (k + 1), ts512])
                    xts.append(xt)
                # qT / kT  (m 0..2 -> q pairs, 3..5 -> k pairs)
                for m in range(6):
                    ps = psgp.tile([128, 512], f32, tag="sg")
                    for k in range(KT):
                        nc.tensor.matmul(ps[:], wqk[k][:, 128 * m:128 * (m + 1)],
                                         xts[k][:],
                                         start=(k == 0), stop=(k == KT - 1))
                    dest = qT[m] if m < 3 else kT[m - 3]
                    nc.vector.tensor_scalar_add(dest[:, ts512], ps[:], bqk[m][:])
                # v natural layout
                for tsub in range(4):
                    jt = 4 * tci + tsub
                    ps = psgp.tile([128, 384], f32, tag="sg")
                    for k in range(KT):
                        nc.tensor.matmul(
                            ps[:],
                            xts[k][:, 128 * tsub:128 * (tsub + 1)],
                            wv[k][:],
                            start=(k == 0), stop=(k == KT - 1))
                    nc.vector.tensor_tensor(
                        v[jt][:, :, 0:64],
                        ps[:].rearrange("p (h d) -> p h d", h=6),
                        bvb[:].rearrange("p (h d) -> p h d", h=6),
                        op=ALU.add)

            if dbg:
                nc.sync.dma_start(dbg_qT.ap()[:], qT[0][:])
                nc.sync.dma_start(dbg_kT.ap()[:], kT[0][:])
                nc.sync.dma_start(dbg_v.ap()[:], v[0][:].rearrange("p h d -> p (h d)"))

            # ---------------- late constants (mask, proj weights/bias) ----
            msk = constp.tile([128, 4, 512], f32r, tag="msk")
            nc.sync.dma_start(msk[:], mask_d.ap()[:])
            wp = []
            for m in range(NP):
                t = constp.tile([128, 768], f32r, tag=f"wp{m}")
                nc.sync.dma_start(t[:], wp_d.ap()[128 * m:128 * (m + 1), :])
                wp.append(t)
            bp_row = constp.tile([1, 768], f32r, tag="bpr")
            nc.sync.dma_start(bp_row[:], bp_d.ap()[:])
            bpb = constp.tile([128, 768], f32, tag="bpb")
            ps = psgp.tile([128, 768], f32, tag="sg")
            for lo, hi in [(0, 512), (512, 768)]:
                nc.tensor.matmul(ps[:, lo:hi], ones[0:1, :],
                                 bp_row[:, lo:hi], start=True, stop=True)
            nc.vector.tensor_copy(bpb[:], ps[:])

            # ---------------- attention + projection ----------------------
            outT = [otp.tile([128, T], f32r, tag=f"outT{p}", name=f"outT{p}") for p in range(NP)]

            for ic in range(NIC):
                isl = slice(512 * ic, 512 * (ic + 1))
                njt = 4 * ic + 4          # j-tiles for this i-chunk (incl diag 4)
                ngr = njt // 2            # score groups of 2 j-tiles
                for p in range(NP):
                    pairs = [(0, slice(0, 64)), (1, slice(64, 128))]
                    # Pipeline per group of 2 j-tiles: scores (row-tiled head
                    # pair) -> exp (+causal mask on the 2 diagonal-straddling
                    # groups) -> AV accumulation (M=65: V plus a ones column,
                    # so row 64 of the psum accumulates the softmax sum S).
                    av = {h: pavp.tile([65, 512], f32, tag="av", name=f"av{h}",
                       bufs=3)
                          for h, _ in pairs}
                    for gi in range(ngr):
                        ets = {}
                        for h, dsl in pairs:
                            sg = psgp.tile([128, 2, 512], f32, tag="sg")
                            for j2 in range(2):
                                jt = 2 * gi + j2
                                nc.tensor.matmul(
                                    sg[:, j2, :],
                                    kT[p][dsl, 128 * jt:128 * (jt + 1)],
                                    qT[p][dsl, isl],
                                    start=True, stop=True,
                                    tile_position=(64 * h, 0))
                            et = esp.tile([128, 2, 512], f32r, tag=f"e{h}")
                            nc.scalar.activation(et[:], sg[:], AF.Exp, scale=SCALE)
                            if gi >= ngr - 2:
                                d0 = 2 * (gi - (ngr - 2))
                                nc.vector.tensor_tensor(
                                    et[:], et[:], msk[:, d0:d0 + 2, :],
                                    op=ALU.mult)
                            ets[h] = et
                        if dbg and p == 0 and ic == min(NIC - 1, 1):
                            if gi == 0:
                                nc.sync.dma_start(
                                    dbg_ef.ap()[:],
                                    ets[0][:].rearrange("p a b -> p (a b)"))
                            if gi == ngr - 1:
                                nc.sync.dma_start(
                                    dbg_ed.ap()[:],
                                    ets[0][:].rearrange("p a b -> p (a b)"))
                        for h, dsl in pairs:
                            hl = 2 * p + h
                            for j2 in range(2):
                                jt = 2 * gi + j2
                                nc.tensor.matmul(
                                    av[h][:, :], v[jt][:, hl, :],
                                    ets[h][:, j2, :],
                                    start=(jt == 0), stop=(jt == njt - 1))
                    if dbg and p == 0 and ic == min(NIC - 1, 1):
                        tdbg = ysp.tile([128, 512], f32, tag="y", name="tdbg")
                        nc.vector.tensor_copy(tdbg[0:64, :], av[0][0:64, :])
                        nc.vector.tensor_copy(tdbg[64:128, :], av[1][0:64, :])
                        nc.sync.dma_start(dbg_av.ap()[:], tdbg[:])

                    # -- normalize: outT_h = av_h[0:64] * (1/S_h) where
                    #    S_h = av_h[64] (ones-column sum); 1/S broadcast to 64
                    #    partitions via a K=1 matmul
                    for h, dsl in pairs:
                        # 1/S = exp(-ln S) on ScalarE (same ACT table set
                        # as the softmax exp; avoids the slow DVE reciprocal)
                        rf = rsp.tile([1, 512], f32, tag="rf")
                        nc.scalar.activation(rf[:], av[h][64:65, :], AF.Ln)
                        rr = rsp.tile([1, 512], f32r, tag="rr")
                        nc.scalar.activation(rr[:], rf[:], AF.Exp, scale=-1.0)
                        rbp = pavp.tile([64, 512], f32, tag="rb", bufs=1)
                        nc.tensor.matmul(rbp[:], ones[0:1, 0:64], rr[:],
                                         start=True, stop=True)
                        rbs = rsp.tile([64, 512], f32, tag="rbs")
                        nc.vector.tensor_copy(rbs[:], rbp[:])
                        if dbg and p == 0 and ic == min(NIC - 1, 1):
                            nc.sync.dma_start(
                                dbg_rb.ap()[64 * h:64 * h + 64, :], rbs[:])
                        nc.vector.tensor_tensor(outT[p][dsl, isl],
                                                av[h][0:64, :], rbs[:],
                                                op=ALU.mult)

                if dbg and ic == NIC - 1:
                    nc.sync.dma_start(dbg_oT.ap()[:], outT[0][:])
                # -- output projection for this i-chunk
                for tsub in range(4):
                    t0 = 512 * ic + 128 * tsub
                    ysb = ysp.tile([128, 768], f32, tag="y")
                    for n in range(2):
                        nsl = slice(384 * n, 384 * (n + 1))
                        yp = pavp.tile([128, 384], f32, tag="rb", bufs=1)
                        for mp in range(NP):
                            nc.tensor.matmul(
                                yp[:], outT[mp][:, t0:t0 + 128],
                                wp[mp][:, nsl],
                                start=(mp == 0), stop=(mp == NP - 1))
                        nc.vector.tensor_tensor(ysb[:, nsl], yp[:], bpb[:, nsl],
                                                op=ALU.add)
                    nc.sync.dma_start(y_d.ap()[t0:t0 + 128, :], ysb[:])

    nc.compile()
    return nc


def make_in_maps(x, w_attn, b_attn, w_proj, b_proj, T=T_FULL):
    x = np.asarray(x, np.float32)
    w_attn = np.asarray(w_attn, np.float32)
    b_attn = np.asarray(b_attn, np.float32)
    w_proj = np.asarray(w_proj, np.float32)
    b_proj = np.asarray(b_proj, np.float32)
    B = x.shape[0]

    ones = np.ones((128, 128), np.float32)
    # mask[jp, d, il] = 1 iff key (128*d + jp) <= query il, for the 4
    # diagonal-straddling j-tiles of each 512-query chunk
    mask = (128 * np.arange(4)[None, :, None] + np.arange(128)[:, None, None]
            <= np.arange(512)[None, None, :]).astype(np.float32)

    in_maps = []
    for c in range(N_CORES):
        b, g = (c // 2) % B, c % 2
        q0, k0, v0 = 384 * g, 768 + 384 * g, 1536 + 384 * g
        wqk = np.concatenate(
            [w_attn[:, q0:q0 + 384], w_attn[:, k0:k0 + 384]], axis=1)
        bqk = np.concatenate(
            [b_attn[q0:q0 + 384], b_attn[k0:k0 + 384]]).reshape(6, 128, 1)
        in_maps.append({
            "xT": np.ascontiguousarray(x[b].T),
            "wqk": np.ascontiguousarray(wqk),
            "wv": np.ascontiguousarray(w_attn[:, v0:v0 + 384]),
            "wp": np.ascontiguousarray(w_proj[384 * g:384 * (g + 1), :]),
            "bqk": np.ascontiguousarray(bqk),
            "bv": np.ascontiguousarray(b_attn[v0:v0 + 384].reshape(1, 384)),
            "bp": np.ascontiguousarray(
                (b_proj if g == 0 else np.zeros_like(b_proj)).reshape(1, -1)),
            "ones": ones,
            "mask": np.ascontiguousarray(mask),
        })
    return in_maps


def kernel(x, w_attn, b_attn, w_proj, b_proj):
    global LAST_RESULT
    if "nc" not in _NC_CACHE:
        _NC_CACHE["nc"] = build_nc(T_FULL)
    nc = _NC_CACHE["nc"]
    in_maps = make_in_maps(x, w_attn, b_attn, w_proj, b_proj)
    res = bass_utils.run_bass_kernel_spmd(
        nc, in_maps, core_ids=list(range(N_CORES)), trace=TRACE)
    LAST_RESULT = res
    B, T, C = np.asarray(x).shape
    y = np.empty((B, T, C), np.float32)
    for b in range(B):
        y[b] = res.results[2 * b]["y"] + res.results[2 * b + 1]["y"]
    return y


# revision 26
# speedup vs baseline: 1.0846x; 1.0846x over previous
"""Causal self-attention (GPT-style, 12 heads, C=768) on 8 TRN2 NeuronCores.

Sharding: core c -> (batch b = c//2, head-group g = c%2 of 6 heads).
Each core computes qkv projection for its 6 heads, causal attention, and a
partial output projection (its 384 rows of w_proj). Host sums the two
partial projections per batch (row-parallel tensor parallelism) and adds
nothing else (b_proj is folded into the g=0 core's partial).

All matmuls run as float32r (full PE rate at N>=256, ~1e-4 precision).
Layouts chosen so no on-device transposes are needed:
  - x is transposed on host -> xT [C, T]
  - qkv matmul produces qT/kT directly ([head-pair d, T]); V in natural [T, d]
  - scores computed transposed: sT[j, i] = K Q^T via lhsT=kT, rhs=qT
  - softmax denominator via ones-vector matmul (S = sum_j exp)
  - out^T [d, i] = V^T exp accumulated in PSUM, normalized by 1/S broadcast
    (broadcast via K=1 matmul), written as outT [384, T] = proj lhsT.
Head-pairs are packed 2-per-128-partitions: QK uses row-tiled concurrent
matmuls (K=64 at base partition 0/64), AV/S use col-tiled concurrent
matmuls (output at partition base 0/64 resp. 0/32).
"""

import numpy as np

import concourse.bass as bass
import concourse.mybir as mybir
import concourse.tile as tile
from concourse import bacc
from concourse import bass_utils

f32 = mybir.dt.float32
f32r = mybir.dt.float32r
AF = mybir.ActivationFunctionType
ALU = mybir.AluOpType

N_HEAD = 12
N_EMBD = 768
B_FULL = 4
T_FULL = 2048
N_CORES = 8
SCALE = float(N_EMBD) ** -0.5

TRACE = False
LAST_RESULT = None
_NC_CACHE = {}


def build_nc(T=T_FULL, dbg=False):
    """Build the per-core Bass program. All 8 cores run this same program
    on different input data."""
    C = N_EMBD            # 768
    HC = 6                # local heads per core
    NP = 3                # head pairs
    D = 64                # head dim
    KT = C // 128         # 6 k-tiles for the projections
    NIC = T // 512        # i-chunks (512 queries each)
    NJT = T // 128        # j-tiles (128 keys each)

    # Force all ACT ops (softmax Exp + the 1/S Ln/Exp pair) onto the one
    # table set that contains both functions, so the activation-table-load
    # pass emits a single load instead of thrashing between sets. Entry
    # order (and hence act_func_set ids) is preserved; we only hide Exp/Ln
    # from the other sets during this build.
    import concourse.bacc as _bacc_mod
    from concourse.hw_specs import get_activation_tables as _orig_gat

    def _pinned_gat(arch):
        tabs = {k: set(v) for k, v in _orig_gat(arch).items()}
        for name, fns in tabs.items():
            if name != "natural_log_exp_and_others":
                fns.discard(AF.Exp)
                fns.discard(AF.Ln)
        return tabs

    nc = bacc.Bacc("TRN2", target_bir_lowering=False, debug=False)

    xT_d = nc.dram_tensor("xT", [C, T], f32r, kind="ExternalInput")
    wqk_d = nc.dram_tensor("wqk", [C, 768], f32r, kind="ExternalInput")
    wv_d = nc.dram_tensor("wv", [C, 384], f32r, kind="ExternalInput")
    wp_d = nc.dram_tensor("wp", [384, C], f32r, kind="ExternalInput")
    bqk_d = nc.dram_tensor("bqk", [6, 128, 1], f32, kind="ExternalInput")
    bv_d = nc.dram_tensor("bv", [1, 384], f32r, kind="ExternalInput")
    bp_d = nc.dram_tensor("bp", [1, C], f32r, kind="ExternalInput")
    ones_d = nc.dram_tensor("ones", [128, 128], f32r, kind="ExternalInput")
    mask_d = nc.dram_tensor("mask", [128, 4, 512], f32r, kind="ExternalInput")
    y_d = nc.dram_tensor("y", [T, C], f32, kind="ExternalOutput")
    if dbg:
        dbg_qT = nc.dram_tensor("dbg_qT", [128, T], f32r, kind="ExternalOutput")
        dbg_kT = nc.dram_tensor("dbg_kT", [128, T], f32r, kind="ExternalOutput")
        dbg_v = nc.dram_tensor("dbg_v", [128, 390], f32r, kind="ExternalOutput")
        dbg_oT = nc.dram_tensor("dbg_oT", [128, T], f32r, kind="ExternalOutput")
        dbg_ef = nc.dram_tensor("dbg_ef", [128, 1024], f32r, kind="ExternalOutput")
        dbg_ed = nc.dram_tensor("dbg_ed", [128, 512], f32r, kind="ExternalOutput")
        dbg_av = nc.dram_tensor("dbg_av", [128, 512], f32, kind="ExternalOutput")
        dbg_s = nc.dram_tensor("dbg_s", [2, 512], f32, kind="ExternalOutput")
        dbg_rb = nc.dram_tensor("dbg_rb", [128, 512], f32, kind="ExternalOutput")

    with tile.TileContext(nc) as tc:
        with (
            tc.tile_pool(name="const", bufs=1) as constp,
            tc.tile_pool(name="xt", bufs=8) as xtp,
            tc.tile_pool(name="qk", bufs=1) as qkp,
            tc.tile_pool(name="vs", bufs=16) as vsp,
            tc.tile_pool(name="es", bufs=3) as esp,
            tc.tile_pool(name="ot", bufs=1) as otp,
            tc.tile_pool(name="ys", bufs=2) as ysp,
            tc.tile_pool(name="rs", bufs=1) as rsp,
            tc.tile_pool(name="psg", bufs=2, space="PSUM") as psgp,
            tc.tile_pool(name="pav", bufs=2, space="PSUM") as pavp,
        ):
            # ---------------- setup: only what the qkv phase needs -------
            ones = constp.tile([128, 128], f32r, tag="ones")
            nc.sync.dma_start(ones[:], ones_d.ap()[:])
            wqk = []
            wv = []
            for k in range(KT):
                t = constp.tile([128, 768], f32r, tag=f"wqk{k}")
                nc.sync.dma_start(t[:], wqk_d.ap()[128 * k:128 * (k + 1), :])
                wqk.append(t)
                t = constp.tile([128, 384], f32r, tag=f"wv{k}")
                nc.sync.dma_start(t[:], wv_d.ap()[128 * k:128 * (k + 1), :])
                wv.append(t)
            bqk = []
            for m in range(6):
                t = constp.tile([128, 1], f32, tag=f"bqk{m}")
                nc.sync.dma_start(t[:], bqk_d.ap()[m, :, :])
                bqk.append(t)
            bv_row = constp.tile([1, 384], f32r, tag="bvr")
            nc.sync.dma_start(bv_row[:], bv_d.ap()[:])
            bvb = constp.tile([128, 384], f32, tag="bvb")
            ps = psgp.tile([128, 384], f32, tag="sg")
            nc.tensor.matmul(ps[:], ones[0:1, :], bv_row[:], start=True, stop=True)
            nc.vector.tensor_copy(bvb[:], ps[:])

            # ---------------- qkv projection ------------------------------
            # qT/kT: [128 (pair dims), T]; v: per t-tile [128 (t), 384]
            qT = [qkp.tile([128, T], f32r, tag=f"qT{p}", name=f"qT{p}") for p in range(NP)]
            kT = [qkp.tile([128, T], f32r, tag=f"kT{p}", name=f"kT{p}") for p in range(NP)]
            v = [vsp.tile([128, 6, 65], f32r, tag="v", name=f"v{j}") for j in range(NJT)]
            for j in range(NJT):
                nc.sync.dma_start(v[j][:, :, 64:65], ones[:, 0:6])

            for tci in range(NIC):
                ts512 = slice(512 * tci, 512 * (tci + 1))
                xts = []
                for k in range(KT):
                    xt = xtp.tile([128, 512], f32r, tag="xt")
                    nc.sync.dma_start(xt[:], xT_d.ap()[128 * k:128 * (k + 1), ts512])
                    xts.append(xt)
                # qT / kT  (m 0..2 -> q pairs, 3..5 -> k pairs)
                for m in range(6):
                    ps = psgp.tile([128, 512], f32, tag="sg")
                    for k in range(KT):
                        nc.tensor.matmul(ps[:], wqk[k][:, 128 * m:128 * (m + 1)],
                                         xts[k][:],
                                         start=(k == 0), stop=(k == KT - 1))
                    dest = qT[m] if m < 3 else kT[m - 3]
                    nc.vector.tensor_scalar_add(dest[:, ts512], ps[:], bqk[m][:])
                # v natural layout
                for tsub in range(4):
                    jt = 4 * tci + tsub
                    ps = psgp.tile([128, 384], f32, tag="sg")
                    for k in range(KT):
                        nc.tensor.matmul(
                            ps[:],
                            xts[k][:, 128 * tsub:128 * (tsub + 1)],
                            wv[k][:],
                            start=(k == 0), stop=(k == KT - 1))
                    nc.vector.tensor_tensor(
                        v[jt][:, :, 0:64],
                        ps[:].rearrange("p (h d) -> p h d", h=6),
                        bvb[:].rearrange("p (h d) -> p h d", h=6),
                        op=ALU.add)

            if dbg:
                nc.sync.dma_start(dbg_qT.ap()[:], qT[0][:])
                nc.sync.dma_start(dbg_kT.ap()[:], kT[0][:])
                nc.sync.dma_start(dbg_v.ap()[:], v[0][:].rearrange("p h d -> p (h d)"))

            # ---------------- late constants (mask, proj weights/bias) ----
            msk = constp.tile([128, 4, 512], f32r, tag="msk")
            nc.sync.dma_start(msk[:], mask_d.ap()[:])
            wp = []
            for m in range(NP):
                t = constp.tile([128, 768], f32r, tag=f"wp{m}")
                nc.sync.dma_start(t[:], wp_d.ap()[128 * m:128 * (m + 1), :])
                wp.append(t)
            bp_row = constp.tile([1, 768], f32r, tag="bpr")
            nc.sync.dma_start(bp_row[:], bp_d.ap()[:])
            bpb = constp.tile([128, 768], f32, tag="bpb")
            ps = psgp.tile([128, 768], f32, tag="sg")
            for lo, hi in [(0, 512), (512, 768)]:
                nc.tensor.matmul(ps[:, lo:hi], ones[0:1, :],
                                 bp_row[:, lo:hi], start=True, stop=True)
            nc.vector.tensor_copy(bpb[:], ps[:])

            # ---------------- attention + projection ----------------------
            outT = [otp.tile([128, T], f32r, tag=f"outT{p}", name=f"outT{p}") for p in range(NP)]

            for ic in range(NIC):
                isl = slice(512 * ic, 512 * (ic + 1))
                njt = 4 * ic + 4          # j-tiles for this i-chunk (incl diag 4)
                ngr = njt // 2            # score groups of 2 j-tiles
                for p in range(NP):
                    pairs = [(0, slice(0, 64)), (1, slice(64, 128))]
                    # Pipeline per group of 2 j-tiles: scores (row-tiled head
                    # pair) -> exp (+causal mask on the 2 diagonal-straddling
                    # groups) -> AV accumulation (M=65: V plus a ones column,
                    # so row 64 of the psum accumulates the softmax sum S).
                    av = {h: pavp.tile([65, 512], f32, tag="av", name=f"av{h}",
                       bufs=3)
                          for h, _ in pairs}
                    for gi in range(ngr):
                        ets = {}
                        for h, dsl in pairs:
                            sg = psgp.tile([128, 2, 512], f32, tag="sg")
                            for j2 in range(2):
                                jt = 2 * gi + j2
                                nc.tensor.matmul(
                                    sg[:, j2, :],
                                    kT[p][dsl, 128 * jt:128 * (jt + 1)],
                                    qT[p][dsl, isl],
                                    start=True, stop=True,
                                    tile_position=(64 * h, 0))
                            et = esp.tile([128, 2, 512], f32r, tag=f"e{h}")
                            nc.scalar.activation(et[:], sg[:], AF.Exp, scale=SCALE)
                            if gi >= ngr - 2:
                                d0 = 2 * (gi - (ngr - 2))
                                nc.vector.tensor_tensor(
                                    et[:], et[:], msk[:, d0:d0 + 2, :],
                                    op=ALU.mult)
                            ets[h] = et
                        if dbg and p == 0 and ic == min(NIC - 1, 1):
                            if gi == 0:
                                nc.sync.dma_start(
                                    dbg_ef.ap()[:],
                                    ets[0][:].rearrange("p a b -> p (a b)"))
                            if gi == ngr - 1:
                                nc.sync.dma_start(
                                    dbg_ed.ap()[:],
                                    ets[0][:].rearrange("p a b -> p (a b)"))
                        for h, dsl in pairs:
                            hl = 2 * p + h
                            for j2 in range(2):
                                jt = 2 * gi + j2
                                nc.tensor.matmul(
                                    av[h][:, :], v[jt][:, hl, :],
                                    ets[h][:, j2, :],
                                    start=(jt == 0), stop=(jt == njt - 1))
                    if dbg and p == 0 and ic == min(NIC - 1, 1):
                        tdbg = ysp.tile([128, 512], f32, tag="y", name="tdbg")
                        nc.vector.tensor_copy(tdbg[0:64, :], av[0][0:64, :])
                        nc.vector.tensor_copy(tdbg[64:128, :], av[1][0:64, :])
                        nc.sync.dma_start(dbg_av.ap()[:], tdbg[:])

                    # -- normalize: outT_h = av_h[0:64] * (1/S_h) where
                    #    S_h = av_h[64] (ones-column sum); 1/S broadcast to 64
                    #    partitions via a K=1 matmul
                    for h, dsl in pairs:
                        # 1/S = exp(-ln S) on ScalarE (same ACT table set
                        # as the softmax exp; avoids the slow DVE reciprocal)
                        rf = rsp.tile([1, 512], f32, tag="rf")
                        nc.scalar.activation(rf[:], av[h][64:65, :], AF.Ln)
                        rr = rsp.tile([1, 512], f32r, tag="rr")
                        nc.scalar.activation(rr[:], rf[:], AF.Exp, scale=-1.0)
                        rbp = pavp.tile([64, 512], f32, tag="rb", bufs=1)
                        nc.tensor.matmul(rbp[:], ones[0:1, 0:64], rr[:],
                                         start=True, stop=True)
                        rbs = rsp.tile([64, 512], f32, tag="rbs")
                        nc.vector.tensor_copy(rbs[:], rbp[:])
                        if dbg and p == 0 and ic == min(NIC - 1, 1):
                            nc.sync.dma_start(
                                dbg_rb.ap()[64 * h:64 * h + 64, :], rbs[:])
                        nc.vector.tensor_tensor(outT[p][dsl, isl],
                                                av[h][0:64, :], rbs[:],
                                                op=ALU.mult)

                if dbg and ic == NIC - 1:
                    nc.sync.dma_start(dbg_oT.ap()[:], outT[0][:])
                # -- output projection for this i-chunk
                for tsub in range(4):
                    t0 = 512 * ic + 128 * tsub
                    ysb = ysp.tile([128, 768], f32, tag="y")
                    for n in range(2):
                        nsl = slice(384 * n, 384 * (n + 1))
                        yp = pavp.tile([128, 384], f32, tag="rb", bufs=1)
                        for mp in range(NP):
                            nc.tensor.matmul(
                                yp[:], outT[mp][:, t0:t0 + 128],
                                wp[mp][:, nsl],
                                start=(mp == 0), stop=(mp == NP - 1))
                        nc.vector.tensor_tensor(ysb[:, nsl], yp[:], bpb[:, nsl],
                                                op=ALU.add)
                    nc.sync.dma_start(y_d.ap()[t0:t0 + 128, :], ysb[:])

    _bacc_mod.get_activation_tables = _pinned_gat
    try:
        nc.compile()
    finally:
        _bacc_mod.get_activation_tables = _orig_gat
    return nc


def make_in_maps(x, w_attn, b_attn, w_proj, b_proj, T=T_FULL):
    x = np.asarray(x, np.float32)
    w_attn = np.asarray(w_attn, np.float32)
    b_attn = np.asarray(b_attn, np.float32)
    w_proj = np.asarray(w_proj, np.float32)
    b_proj = np.asarray(b_proj, np.float32)
    B = x.shape[0]

    ones = np.ones((128, 128), np.float32)
    # mask[jp, d, il] = 1 iff key (128*d + jp) <= query il, for the 4
    # diagonal-straddling j-tiles of each 512-query chunk
    mask = (128 * np.arange(4)[None, :, None] + np.arange(128)[:, None, None]
            <= np.arange(512)[None, None, :]).astype(np.float32)

    in_maps = []
    for c in range(N_CORES):
        b, g = (c // 2) % B, c % 2
        q0, k0, v0 = 384 * g, 768 + 384 * g, 1536 + 384 * g
        wqk = np.concatenate(
            [w_attn[:, q0:q0 + 384], w_attn[:, k0:k0 + 384]], axis=1)
        bqk = np.concatenate(
            [b_attn[q0:q0 + 384], b_attn[k0:k0 + 384]]).reshape(6, 128, 1)
        in_maps.append({
            "xT": np.ascontiguousarray(x[b].T),
            "wqk": np.ascontiguousarray(wqk),
            "wv": np.ascontiguousarray(w_attn[:, v0:v0 + 384]),
            "wp": np.ascontiguousarray(w_proj[384 * g:384 * (g + 1), :]),
            "bqk": np.ascontiguousarray(bqk),
            "bv": np.ascontiguousarray(b_attn[v0:v0 + 384].reshape(1, 384)),
            "bp": np.ascontiguousarray(
                (b_proj if g == 0 else np.zeros_like(b_proj)).reshape(1, -1)),
            "ones": ones,
            "mask": np.ascontiguousarray(mask),
        })
    return in_maps


def kernel(x, w_attn, b_attn, w_proj, b_proj):
    global LAST_RESULT
    if "nc" not in _NC_CACHE:
        _NC_CACHE["nc"] = build_nc(T_FULL)
    nc = _NC_CACHE["nc"]
    in_maps = make_in_maps(x, w_attn, b_attn, w_proj, b_proj)
    res = bass_utils.run_bass_kernel_spmd(
        nc, in_maps, core_ids=list(range(N_CORES)), trace=TRACE)
    LAST_RESULT = res
    B, T, C = np.asarray(x).shape
    y = np.empty((B, T, C), np.float32)
    for b in range(B):
        y[b] = res.results[2 * b]["y"] + res.results[2 * b + 1]["y"]
    return y
